# revision 12
# baseline (speedup 1.0000x reference)
"""Locally-connected layer (unshared 3x3 conv, torch-unfold semantics) on 8 trn2 cores.

out[b,o,y,x] = sum_{c,i,j} weight[o, c*9+i*3+j, y*32+x] * xpad[b, c, y+i, x+j] + bias[o, l]

Sharding: spatial over L - core r owns image rows [4r, 4r+4) (128 pixels).

Design history:
  v1 (385 us): fp32, per-pixel N=64 matmuls, strided weight DMA -> 128 B
      packets, 10% memory-bandwidth utilization, DMA-bound.
  v2/v3 (63/69 us): bf16 everywhere + host relayout of weights into the SBUF
      stream layout (contiguous 6-18 KB DMA packets). Remaining losses: the
      K=128 fused matmuls occupy all PE rows so every LDWEIGHTS serializes
      with the matmul stream (~110 ns per matmul slot), plus a 12 us head.
  v4 (HW crash): K=64 matmuls alternating PE row groups per block, single
      psum group mixing base-0/base-64 -> TRN2 erratum ("matmul groups that
      mix 64-row blocks at base 0 and base 64 crash", bisected earlier).
  v5 (verifier reject): two psum banks merged by tensor_tensor(psum, psum).
  v6: COLUMN-LEVEL row-group alternation. Even output columns run their 18
      K=64 matmuls entirely on PE rows 0:64 (SBUF partitions 0:64) into psum
      bank A; odd columns entirely on rows 64:128 into bank B. The two
      columns' matmuls interleave in issue order, so while one group's
      matmul streams the other group's LDWEIGHTS loads, and the matmuls
      themselves execute concurrently on disjoint PE row halves. Each psum
      bank sees one homogeneous accumulation group (the HW-validated
      row-tiling pattern), and the result needs only the plain DVE cast.

Per output column: 18 matmuls (6 slab rows x 3 kernel cols, N = vi*64 <= 192,
K=64) into one psum bank [B=64, 4y*64o=256].

x slab [p, xs, rp, b]: partitions 0:64 hold the zero-padded slab from HBM;
partitions 64:128 get an SBUF->SBUF copy (saves 1.67 MB of HBM traffic).
Weights are host-packed into WG[128, 16, 2304]: partition half = column
parity, so each DMA packet is per-partition contiguous.
Weights ride the sync HWDGE ring; x, the x-copy, and outputs ride the scalar
ring. Small first chunks minimize head latency; 4-pair chunks give 18 KB
packets.

Per-core HBM: w 9.44 MB + x 1.67 MB + out 1.05 MB = 12.2 MB bf16.
"""

import numpy as np

B, C, O, H, W, KS = 64, 64, 64, 32, 32, 3
L = H * W
NCORES = 8
RPC = H // NCORES            # image rows per core = 4
SLABR = RPC + 2              # slab rows per core (with halo) = 6
XS = W + 2                   # padded slab width = 34
NPAIR = W // 2               # 16 column pairs

# output rows y served by slab row rp: y = rp - i, i in 0..2, clipped
YS = [max(0, rp - 2) for rp in range(SLABR)]
VI = [min(RPC - 1, rp) - max(0, rp - 2) + 1 for rp in range(SLABR)]

# per-column block list: (rp, j, ya, n, off); off = cumulative stream column
BLOCKS = []
_off = 0
for _rp in range(SLABR):
    for _j in range(KS):
        _n = VI[_rp] * O
        BLOCKS.append((_rp, _j, YS[_rp] * O, _n, _off))
        _off += _n
CCOLS = _off                 # 2304 weight stream cols per output column
assert CCOLS == 2304

_CACHE = {}


def _build_nc():
    import concourse.bass as bass
    import concourse.bacc as bacc
    import concourse.tile as tile
    from concourse import mybir

    f32 = mybir.dt.float32
    bf16 = mybir.dt.bfloat16
    nc = bacc.Bacc(
        "TRN2", target_bir_lowering=False, debug=False, num_devices=NCORES
    )
    # x slab (lower half only; upper half is an on-chip copy): [c, xs, rp, b]
    x_d = nc.dram_tensor("xf", [64, XS, SLABR, B], bf16, kind="ExternalInput")
    # weight stream: [p, pair, ccol]; p<64 -> even column, p>=64 -> odd
    wg_d = nc.dram_tensor("wg", [128, NPAIR, CCOLS], bf16, kind="ExternalInput")
    # out [b, x, (y, o)]
    o_d = nc.dram_tensor("out", [B, W, RPC * O], bf16, kind="ExternalOutput")

    # chunks in column-pair units; small first chunks minimize head latency
    XCH = [(0, 1), (1, 2), (2, 4), (4, 8), (8, 12), (12, 16)]
    # x-slab cols needed for pair chunk [a, b): xs in [2a, 2b+2)
    XFCH = [(0, 4), (4, 6), (6, 10), (10, 18), (18, 26), (26, 34)]

    with tile.TileContext(nc) as tc:
        with (
            tc.tile_pool(name="xp", bufs=1) as xpool,
            tc.tile_pool(name="wg", bufs=4) as wgpool,
            tc.tile_pool(name="ot", bufs=2) as opool,
            tc.tile_pool(name="ps", bufs=3, space=bass.MemorySpace.PSUM) as pspool,
        ):
            xf = xpool.tile([128, XS, SLABR, B], bf16)
            for (a, b), (fa, fb) in zip(XCH, XFCH):
                # x chunk + its upper-half copy on the scalar ring,
                # weights on the sync ring (consumption order per ring)
                nc.scalar.dma_start(xf[0:64, fa:fb], x_d[:, fa:fb])
                nc.scalar.dma_start(xf[64:128, fa:fb], xf[0:64, fa:fb])
                nch = b - a
                wgk = wgpool.tile([128, nch, CCOLS], bf16,
                                  name=f"wgk{a}", tag="wgk")
                nc.sync.dma_start(wgk[:], wg_d[:, a:b])
                ot = opool.tile([B, 2 * nch, RPC * O], bf16,
                                name=f"ot{a}", tag="ot")

                for p in range(a, b):
                    pi = p - a
                    # even column -> PE rows 0:64 / bank A; odd column ->
                    # rows 64:128 / bank B. Interleaved issue order lets
                    # LDWEIGHTS and matmuls of the two groups overlap.
                    ps0 = pspool.tile([B, RPC * O], f32, name="psA", tag="psA")
                    ps1 = pspool.tile([B, RPC * O], f32, name="psB", tag="psB")
                    last = len(BLOCKS) - 1
                    for k, (rp, j, ya, n, off) in enumerate(BLOCKS):
                        nc.tensor.matmul(
                            ps0[:, ya : ya + n],
                            xf[0:64, 2 * p + j, rp, :],
                            wgk[0:64, pi, off : off + n],
                            start=(k == 0), stop=(k == last),
                        )
                        nc.tensor.matmul(
                            ps1[:, ya : ya + n],
                            xf[64:128, 2 * p + 1 + j, rp, :],
                            wgk[64:128, pi, off : off + n],
                            start=(k == 0), stop=(k == last),
                        )
                    nc.vector.tensor_copy(ot[:, 2 * pi, :], ps0[:])
                    nc.vector.tensor_copy(ot[:, 2 * pi + 1, :], ps1[:])
                nc.scalar.dma_start(o_d[:, 2 * a : 2 * b], ot[:])
    nc.compile()
    return nc


def _get_nc():
    if "nc" not in _CACHE:
        _CACHE["nc"] = _build_nc()
    return _CACHE["nc"]


def _shard_inputs(x, weight):
    from concourse import mybir

    bf16 = mybir.dt.np(mybir.dt.bfloat16)

    xpad = np.pad(x, ((0, 0), (0, 0), (1, 1), (1, 1)))  # (B, C, 34, 34)
    XF = np.zeros((NCORES, 64, XS, SLABR, B), np.float32)
    base = xpad.transpose(1, 3, 2, 0)  # (c, col, row, b)
    for rp in range(SLABR):
        # slab row rp of core r is padded row 4r+rp (8 cores)
        XF[:, :, :, rp, :] = (
            base[:, :, rp : rp + 4 * NCORES : 4, :].transpose(2, 0, 1, 3)
        )

    # weight stream; w6[o, c, i, j, y_img, x]
    w6 = weight.reshape(O, C, KS, KS, H, W)
    WG = np.zeros((NCORES, 128, NPAIR, CCOLS), np.float32)
    for rp, j, ya, n, off in BLOCKS:
        for yi in range(VI[rp]):
            y = YS[rp] + yi
            i = rp - y
            csl = slice(off + yi * O, off + (yi + 1) * O)
            # (O, C, R, W) -> (R, C, W, O); even cols to 0:64, odd to 64:128
            src = w6[:, :, i, j, y::RPC, :].transpose(2, 1, 3, 0)
            WG[:, 0:64, :, csl] = src[:, :, 0::2, :]
            WG[:, 64:128, :, csl] = src[:, :, 1::2, :]

    XF = XF.astype(bf16)
    WG = WG.astype(bf16)
    return [{"xf": XF[r], "wg": WG[r]} for r in range(NCORES)]


def kernel(x, weight, bias, _trace=False, _trace_kwargs=None):
    from concourse.bass_utils import run_bass_kernel_spmd

    x = np.ascontiguousarray(np.asarray(x, dtype=np.float32))
    weight = np.asarray(weight, dtype=np.float32)
    bias = np.asarray(bias, dtype=np.float32)

    nc = _get_nc()
    in_maps = _shard_inputs(x, weight)
    res = run_bass_kernel_spmd(
        nc, in_maps, list(range(NCORES)),
        trace=_trace, **(_trace_kwargs or {}),
    )
    # per-core out [B, W, RPC*O] (b, x, y*64+o) -> (B, O, y, x)
    rows = [
        np.asarray(res.results[r]["out"], dtype=np.float32)
        .reshape(B, W, RPC, O)
        .transpose(0, 3, 2, 1)
        for r in range(NCORES)
    ]
    out = np.concatenate(rows, axis=2)  # (B, O, H, W)
    if np.any(bias):
        out = out + bias.reshape(1, O, H, W)
    if _trace:
        _CACHE["last_result"] = res
    return np.ascontiguousarray(out.astype(np.float32))


# revision 14
# speedup vs baseline: 1.1818x; 1.1818x over previous
"""Locally-connected layer (unshared 3x3 conv, torch-unfold semantics) on 8 trn2 cores.

out[b,o,y,x] = sum_{c,i,j} weight[o, c*9+i*3+j, y*32+x] * xpad[b, c, y+i, x+j] + bias[o, l]

Sharding: spatial over L - core r owns image rows [4r, 4r+4) (128 pixels).

Design history:
  v1 (385 us): fp32, per-pixel N=64 matmuls, strided weight DMA -> 128 B
      packets, 10% memory-bandwidth utilization, DMA-bound.
  v2/v3 (63/69 us): bf16 + host relayout of weights into the SBUF stream
      layout (contiguous 6-18 KB DMA packets). K=128 matmuls occupy all PE
      rows -> every LDWEIGHTS serializes with the matmul stream.
  v4 (HW crash): K=64 alternating row groups, one psum group mixing base-0
      and base-64 blocks -> TRN2 erratum (mixed-base groups crash).
  v5 (verifier reject): walrus refuses tensor_tensor with two PSUM inputs.
  v6 (72 us): column-parity row groups, homogeneous psum groups. Correct,
      but the two row groups share the single moving-operand XBUS, so
      matmul streams serialize anyway; 576 matmuls x ~36 ns fixed overhead
      + 30.7 us of K=64 streaming. Also 179 us of summed DMA-engine idle
      right after each DMA's completion semaphore (24 dma_starts).
  v7: 64x64 ARRAY TILING, 4 independent tiles (T0/T2/T8/T10 = SBUF half x
      PSUM half). Four output columns in flight, one per tile, matmuls
      issued round-robin: column tiling allocates separate XBUSes so the
      4 streams + their LDWEIGHTS genuinely overlap. Each column's 18
      K=64 matmuls (N = vi*64 <= 192) form a homogeneous accumulation
      group in its own PSUM bank. PSUM evacuation splits across VectorE
      (lower half) and ScalarE (upper half). Fewer, larger DMAs
      (all weight packets >= 9 KB) to cut the per-dma_start completion
      bubbles; weights on the sync HWDGE ring, x and outputs on scalar.

Column-to-tile map per group of 4 columns (4k..4k+3):
  col 4k   -> T0: stationary+weights SBUF 0:64,    psum partitions 0:64
  col 4k+1 -> T2: SBUF 0:64,    psum 64:128
  col 4k+2 -> T8: SBUF 64:128,  psum 0:64
  col 4k+3 -> T10: SBUF 64:128, psum 64:128
tile_position is auto-derived from (lhsT.base_partition, out.base_partition).

x slab [p, xs, rp, b]: both partition halves hold the zero-padded slab
(host-duplicated; upper half feeds T8/T10). Weights host-packed into
WG[128, 8, 2, 2304]: [sbuf half, group, in-half index, stream col].
Out is staged [128=(parity, b), group, i, 256] and DMAed per half to
o_d[2, B, 16, 256] (parity-split columns), unscrambled on the host.

Per-core HBM: w 9.44 MB + x 3.34 MB + out 1.05 MB = 13.8 MB bf16.
"""

import numpy as np

B, C, O, H, W, KS = 64, 64, 64, 32, 32, 3
L = H * W
NCORES = 8
RPC = H // NCORES            # image rows per core = 4
SLABR = RPC + 2              # slab rows per core (with halo) = 6
XS = W + 2                   # padded slab width = 34
NG = W // 4                  # 8 groups of 4 columns

# output rows y served by slab row rp: y = rp - i, i in 0..2, clipped
YS = [max(0, rp - 2) for rp in range(SLABR)]
VI = [min(RPC - 1, rp) - max(0, rp - 2) + 1 for rp in range(SLABR)]

# per-column block list: (rp, j, ya, n, off); off = cumulative stream column
BLOCKS = []
_off = 0
for _rp in range(SLABR):
    for _j in range(KS):
        _n = VI[_rp] * O
        BLOCKS.append((_rp, _j, YS[_rp] * O, _n, _off))
        _off += _n
CCOLS = _off                 # 2304 weight stream cols per output column
assert CCOLS == 2304

# s = column slot in its group of 4: (sbuf half, psum half)
SHALF = [0, 0, 1, 1]
PHALF = [0, 1, 0, 1]

_CACHE = {}


def _build_nc():
    import concourse.bass as bass
    import concourse.bacc as bacc
    import concourse.tile as tile
    from concourse import mybir

    f32 = mybir.dt.float32
    bf16 = mybir.dt.bfloat16
    nc = bacc.Bacc(
        "TRN2", target_bir_lowering=False, debug=False, num_devices=NCORES
    )
    # x slab duplicated on both partition halves: [p, xs, rp, b]
    x_d = nc.dram_tensor("xf", [128, XS, SLABR, B], bf16, kind="ExternalInput")
    # weight stream: [p, group, i, ccol]; col = 4*group + 2*(p>=64) + i
    wg_d = nc.dram_tensor("wg", [128, NG, 2, CCOLS], bf16, kind="ExternalInput")
    # out, parity-split: [parity, b, m, (y, o)] for col x = 2m + parity
    o_d = nc.dram_tensor("out", [2, B, W // 2, RPC * O], bf16,
                         kind="ExternalOutput")

    # chunks in 4-column-group units; small first chunks cut head latency
    GCH = [(0, 1), (1, 2), (2, 4), (4, 6), (6, 8)]
    # x-slab cols needed for group chunk [a, b): xs in [4a, 4b+2)
    XFCH = [(0, 10), (10, 22), (22, 34)]

    with tile.TileContext(nc) as tc:
        with (
            tc.tile_pool(name="xp", bufs=1) as xpool,
            tc.tile_pool(name="wg", bufs=3) as wgpool,
            tc.tile_pool(name="ot", bufs=2) as opool,
            tc.tile_pool(name="ps", bufs=2, space=bass.MemorySpace.PSUM) as pspool,
        ):
            xf = xpool.tile([128, XS, SLABR, B], bf16)
            for fa, fb in XFCH:
                nc.scalar.dma_start(xf[:, fa:fb], x_d[:, fa:fb])

            for a, b in GCH:
                nch = b - a
                wgk = wgpool.tile([128, nch, 2, CCOLS], bf16,
                                  name=f"wgk{a}", tag="wgk")
                nc.sync.dma_start(wgk[:], wg_d[:, a:b])
                ot = opool.tile([128, nch, 2, RPC * O], bf16,
                                name=f"ot{a}", tag="ot")

                for g in range(a, b):
                    gi = g - a
                    pss = [
                        pspool.tile([128, RPC * O], f32,
                                    name=f"ps{s}", tag=f"ps{s}")
                        for s in range(4)
                    ]
                    last = len(BLOCKS) - 1
                    for k, (rp, j, ya, n, off) in enumerate(BLOCKS):
                        for s in range(4):
                            sh = slice(64 * SHALF[s], 64 * SHALF[s] + 64)
                            ph = slice(64 * PHALF[s], 64 * PHALF[s] + 64)
                            nc.tensor.matmul(
                                pss[s][ph, ya : ya + n],
                                xf[sh, 4 * g + s + j, rp, :],
                                wgk[sh, gi, s % 2, off : off + n],
                                start=(k == 0), stop=(k == last),
                            )
                    # evacuate: lower psum halves on VectorE, upper on
                    # ScalarE; ot partitions = column parity
                    nc.vector.tensor_copy(ot[0:64, gi, 0, :], pss[0][0:64, :])
                    nc.scalar.copy(ot[64:128, gi, 0, :], pss[1][64:128, :])
                    nc.vector.tensor_copy(ot[0:64, gi, 1, :], pss[2][0:64, :])
                    nc.scalar.copy(ot[64:128, gi, 1, :], pss[3][64:128, :])
                # cols of chunk: even = {4g+2i}, odd = {4g+2i+1}; m = 2g+i
                nc.scalar.dma_start(o_d[0, :, 2 * a : 2 * b], ot[0:64])
                nc.scalar.dma_start(o_d[1, :, 2 * a : 2 * b], ot[64:128])
    nc.compile()
    return nc


def _get_nc():
    if "nc" not in _CACHE:
        _CACHE["nc"] = _build_nc()
    return _CACHE["nc"]


def _shard_inputs(x, weight):
    from concourse import mybir

    bf16 = mybir.dt.np(mybir.dt.bfloat16)

    xpad = np.pad(x, ((0, 0), (0, 0), (1, 1), (1, 1)))  # (B, C, 34, 34)
    XF = np.zeros((NCORES, 128, XS, SLABR, B), np.float32)
    base = xpad.transpose(1, 3, 2, 0)  # (c, col, row, b)
    for rp in range(SLABR):
        # slab row rp of core r is padded row 4r+rp (8 cores)
        XF[:, 0:64, :, rp, :] = (
            base[:, :, rp : rp + 4 * NCORES : 4, :].transpose(2, 0, 1, 3)
        )
    XF[:, 64:128] = XF[:, 0:64]

    # weight stream; w6[o, c, i, j, y_img, x]
    w6 = weight.reshape(O, C, KS, KS, H, W)
    WG = np.zeros((NCORES, 128, NG, 2, CCOLS), np.float32)
    for rp, j, ya, n, off in BLOCKS:
        for yi in range(VI[rp]):
            y = YS[rp] + yi
            i = rp - y
            csl = slice(off + yi * O, off + (yi + 1) * O)
            # (O, C, R, W) -> (R, C, W, O); W -> (group, half, i2)
            src = (
                w6[:, :, i, j, y::RPC, :].transpose(2, 1, 3, 0)
                .reshape(NCORES, C, NG, 2, 2, O)
            )
            WG[:, 0:64, :, 0, csl] = src[:, :, :, 0, 0]   # col 4g
            WG[:, 0:64, :, 1, csl] = src[:, :, :, 0, 1]   # col 4g+1
            WG[:, 64:128, :, 0, csl] = src[:, :, :, 1, 0]  # col 4g+2
            WG[:, 64:128, :, 1, csl] = src[:, :, :, 1, 1]  # col 4g+3
    XF = XF.astype(bf16)
    WG = WG.astype(bf16)
    return [{"xf": XF[r], "wg": WG[r]} for r in range(NCORES)]


def kernel(x, weight, bias, _trace=False, _trace_kwargs=None):
    from concourse.bass_utils import run_bass_kernel_spmd

    x = np.ascontiguousarray(np.asarray(x, dtype=np.float32))
    weight = np.asarray(weight, dtype=np.float32)
    bias = np.asarray(bias, dtype=np.float32)

    nc = _get_nc()
    in_maps = _shard_inputs(x, weight)
    res = run_bass_kernel_spmd(
        nc, in_maps, list(range(NCORES)),
        trace=_trace, **(_trace_kwargs or {}),
    )
    # per-core out [2, B, 16, 256]: col x = 2m + parity -> (B, O, y, x)
    rows = []
    for r in range(NCORES):
        o = np.asarray(res.results[r]["out"], dtype=np.float32)
        full = np.zeros((B, W, RPC, O), np.float32)
        full[:, 0::2] = o[0].reshape(B, W // 2, RPC, O)
        full[:, 1::2] = o[1].reshape(B, W // 2, RPC, O)
        rows.append(full.transpose(0, 3, 2, 1))  # (B, O, y, x)
    out = np.concatenate(rows, axis=2)  # (B, O, H, W)
    if np.any(bias):
        out = out + bias.reshape(1, O, H, W)
    if _trace:
        _CACHE["last_result"] = res
    return np.ascontiguousarray(out.astype(np.float32))


# revision 15
# speedup vs baseline: 1.1910x; 1.0078x over previous
"""Locally-connected layer (unshared 3x3 conv, torch-unfold semantics) on 8 trn2 cores.

out[b,o,y,x] = sum_{c,i,j} weight[o, c*9+i*3+j, y*32+x] * xpad[b, c, y+i, x+j] + bias[o, l]

Sharding: spatial over L - core r owns image rows [4r, 4r+4) (128 pixels).

Design history:
  v1 (385 us): fp32, per-pixel N=64 matmuls, strided weight DMA -> 128 B
      packets, 10% memory-bandwidth utilization, DMA-bound.
  v2/v3 (63/69 us): bf16 + host relayout of weights into the SBUF stream
      layout (contiguous 6-18 KB DMA packets). K=128 matmuls occupy all PE
      rows -> every LDWEIGHTS serializes with the matmul stream.
  v4 (HW crash): K=64 alternating row groups, one psum group mixing base-0
      and base-64 blocks -> TRN2 erratum (mixed-base groups crash).
  v5 (verifier reject): walrus refuses tensor_tensor with two PSUM inputs.
  v6 (72 us): column-parity row groups, homogeneous psum groups. Correct,
      but the two row groups share the single moving-operand XBUS, so
      matmul streams serialize anyway; 576 matmuls x ~36 ns fixed overhead
      + 30.7 us of K=64 streaming. Also 179 us of summed DMA-engine idle
      right after each DMA's completion semaphore (24 dma_starts).
  v7: 64x64 ARRAY TILING, 4 independent tiles (T0/T2/T8/T10 = SBUF half x
      PSUM half). Four output columns in flight, one per tile, matmuls
      issued round-robin: column tiling allocates separate XBUSes so the
      4 streams + their LDWEIGHTS genuinely overlap. Each column's 18
      K=64 matmuls (N = vi*64 <= 192) form a homogeneous accumulation
      group in its own PSUM bank. PSUM evacuation splits across VectorE
      (lower half) and ScalarE (upper half). Fewer, larger DMAs
      (all weight packets >= 9 KB) to cut the per-dma_start completion
      bubbles; weights on the sync HWDGE ring, x and outputs on scalar.

Column-to-tile map per group of 4 columns (4k..4k+3):
  col 4k   -> T0: stationary+weights SBUF 0:64,    psum partitions 0:64
  col 4k+1 -> T2: SBUF 0:64,    psum 64:128
  col 4k+2 -> T8: SBUF 64:128,  psum 0:64
  col 4k+3 -> T10: SBUF 64:128, psum 64:128
tile_position is auto-derived from (lhsT.base_partition, out.base_partition).

x slab [p, xs, rp, b]: both partition halves hold the zero-padded slab
(host-duplicated; upper half feeds T8/T10). Weights host-packed into
WG[128, 8, 2, 2304]: [sbuf half, group, in-half index, stream col].
Out is staged [128=(parity, b), group, i, 256] and DMAed per half to
o_d[2, B, 16, 256] (parity-split columns), unscrambled on the host.

Per-core HBM: w 9.44 MB + x 3.34 MB + out 1.05 MB = 13.8 MB bf16.
"""

import numpy as np

B, C, O, H, W, KS = 64, 64, 64, 32, 32, 3
L = H * W
NCORES = 8
RPC = H // NCORES            # image rows per core = 4
SLABR = RPC + 2              # slab rows per core (with halo) = 6
XS = W + 2                   # padded slab width = 34
NG = W // 4                  # 8 groups of 4 columns

# output rows y served by slab row rp: y = rp - i, i in 0..2, clipped
YS = [max(0, rp - 2) for rp in range(SLABR)]
VI = [min(RPC - 1, rp) - max(0, rp - 2) + 1 for rp in range(SLABR)]

# per-column block list: (rp, j, ya, n, off); off = cumulative stream column
BLOCKS = []
_off = 0
for _rp in range(SLABR):
    for _j in range(KS):
        _n = VI[_rp] * O
        BLOCKS.append((_rp, _j, YS[_rp] * O, _n, _off))
        _off += _n
CCOLS = _off                 # 2304 weight stream cols per output column
assert CCOLS == 2304

# s = column slot in its group of 4: (sbuf half, psum half)
SHALF = [0, 0, 1, 1]
PHALF = [0, 1, 0, 1]

_CACHE = {}


def _build_nc():
    import concourse.bass as bass
    import concourse.bacc as bacc
    import concourse.tile as tile
    from concourse import mybir

    f32 = mybir.dt.float32
    bf16 = mybir.dt.bfloat16
    nc = bacc.Bacc(
        "TRN2", target_bir_lowering=False, debug=False, num_devices=NCORES
    )
    # x slab duplicated on both partition halves: [p, xs, rp, b]
    x_d = nc.dram_tensor("xf", [128, XS, SLABR, B], bf16, kind="ExternalInput")
    # weight stream: [p, group, i, ccol]; col = 4*group + 2*(p>=64) + i
    wg_d = nc.dram_tensor("wg", [128, NG, 2, CCOLS], bf16, kind="ExternalInput")
    # out, parity-split: [parity, b, m, (y, o)] for col x = 2m + parity
    o_d = nc.dram_tensor("out", [2, B, W // 2, RPC * O], bf16,
                         kind="ExternalOutput")

    with tile.TileContext(nc) as tc:
        with (
            tc.tile_pool(name="xp", bufs=1) as xpool,
            tc.tile_pool(name="wg", bufs=1) as wgpool,
            tc.tile_pool(name="ot", bufs=2) as opool,
            tc.tile_pool(name="ps", bufs=2, space=bass.MemorySpace.PSUM) as pspool,
        ):
            # All 8 weight-group tiles are resident (74 KB/partition total
            # with xf), so no pool cycling: every input DMA is emitted
            # up-front in consumption order, ping-ponged across the two
            # HWDGE rings. No input-side semaphore stalls -> the SDMA
            # engines never drain. Group 0's weights are split so the
            # first matmuls only wait on 0.3 MB.
            xf = xpool.tile([128, XS, SLABR, B], bf16)
            wgs = [
                wgpool.tile([128, 1, 2, CCOLS], bf16, name=f"wgk{g}", tag=f"w{g}")
                for g in range(NG)
            ]
            SPLIT = 576
            nc.sync.dma_start(xf[:, 0:6], x_d[:, 0:6])
            nc.scalar.dma_start(wgs[0][:, :, :, 0:SPLIT],
                                wg_d[:, 0:1, :, 0:SPLIT])
            nc.sync.dma_start(wgs[0][:, :, :, SPLIT:], wg_d[:, 0:1, :, SPLIT:])
            nc.scalar.dma_start(wgs[1][:], wg_d[:, 1:2])
            nc.sync.dma_start(xf[:, 6:18], x_d[:, 6:18])
            nc.scalar.dma_start(wgs[2][:], wg_d[:, 2:3])
            nc.sync.dma_start(wgs[3][:], wg_d[:, 3:4])
            nc.scalar.dma_start(wgs[4][:], wg_d[:, 4:5])
            nc.sync.dma_start(xf[:, 18:34], x_d[:, 18:34])
            nc.scalar.dma_start(wgs[5][:], wg_d[:, 5:6])
            nc.sync.dma_start(wgs[6][:], wg_d[:, 6:7])
            nc.scalar.dma_start(wgs[7][:], wg_d[:, 7:8])

            for g in range(NG):
                wgk = wgs[g]
                ot = opool.tile([128, 1, 2, RPC * O], bf16,
                                name=f"ot{g}", tag="ot")
                pss = [
                    pspool.tile([128, RPC * O], f32,
                                name=f"ps{s}", tag=f"ps{s}")
                    for s in range(4)
                ]
                last = len(BLOCKS) - 1
                for k, (rp, j, ya, n, off) in enumerate(BLOCKS):
                    for s in range(4):
                        sh = slice(64 * SHALF[s], 64 * SHALF[s] + 64)
                        ph = slice(64 * PHALF[s], 64 * PHALF[s] + 64)
                        nc.tensor.matmul(
                            pss[s][ph, ya : ya + n],
                            xf[sh, 4 * g + s + j, rp, :],
                            wgk[sh, 0, s % 2, off : off + n],
                            start=(k == 0), stop=(k == last),
                        )
                # evacuate: lower psum halves on VectorE, upper on ScalarE;
                # ot partitions = column parity
                nc.vector.tensor_copy(ot[0:64, 0, 0, :], pss[0][0:64, :])
                nc.scalar.copy(ot[64:128, 0, 0, :], pss[1][64:128, :])
                nc.vector.tensor_copy(ot[0:64, 0, 1, :], pss[2][0:64, :])
                nc.scalar.copy(ot[64:128, 0, 1, :], pss[3][64:128, :])
                # cols of group: even = {4g+2i}, odd = {4g+2i+1}; m = 2g+i
                nc.scalar.dma_start(o_d[0, :, 2 * g : 2 * g + 2], ot[0:64])
                nc.scalar.dma_start(o_d[1, :, 2 * g : 2 * g + 2], ot[64:128])
    nc.compile()
    return nc


def _get_nc():
    if "nc" not in _CACHE:
        _CACHE["nc"] = _build_nc()
    return _CACHE["nc"]


def _shard_inputs(x, weight):
    from concourse import mybir

    bf16 = mybir.dt.np(mybir.dt.bfloat16)

    xpad = np.pad(x, ((0, 0), (0, 0), (1, 1), (1, 1)))  # (B, C, 34, 34)
    XF = np.zeros((NCORES, 128, XS, SLABR, B), np.float32)
    base = xpad.transpose(1, 3, 2, 0)  # (c, col, row, b)
    for rp in range(SLABR):
        # slab row rp of core r is padded row 4r+rp (8 cores)
        XF[:, 0:64, :, rp, :] = (
            base[:, :, rp : rp + 4 * NCORES : 4, :].transpose(2, 0, 1, 3)
        )
    XF[:, 64:128] = XF[:, 0:64]

    # weight stream; w6[o, c, i, j, y_img, x]
    w6 = weight.reshape(O, C, KS, KS, H, W)
    WG = np.zeros((NCORES, 128, NG, 2, CCOLS), np.float32)
    for rp, j, ya, n, off in BLOCKS:
        for yi in range(VI[rp]):
            y = YS[rp] + yi
            i = rp - y
            csl = slice(off + yi * O, off + (yi + 1) * O)
            # (O, C, R, W) -> (R, C, W, O); W -> (group, half, i2)
            src = (
                w6[:, :, i, j, y::RPC, :].transpose(2, 1, 3, 0)
                .reshape(NCORES, C, NG, 2, 2, O)
            )
            WG[:, 0:64, :, 0, csl] = src[:, :, :, 0, 0]   # col 4g
            WG[:, 0:64, :, 1, csl] = src[:, :, :, 0, 1]   # col 4g+1
            WG[:, 64:128, :, 0, csl] = src[:, :, :, 1, 0]  # col 4g+2
            WG[:, 64:128, :, 1, csl] = src[:, :, :, 1, 1]  # col 4g+3
    XF = XF.astype(bf16)
    WG = WG.astype(bf16)
    return [{"xf": XF[r], "wg": WG[r]} for r in range(NCORES)]


def kernel(x, weight, bias, _trace=False, _trace_kwargs=None):
    from concourse.bass_utils import run_bass_kernel_spmd

    x = np.ascontiguousarray(np.asarray(x, dtype=np.float32))
    weight = np.asarray(weight, dtype=np.float32)
    bias = np.asarray(bias, dtype=np.float32)

    nc = _get_nc()
    in_maps = _shard_inputs(x, weight)
    res = run_bass_kernel_spmd(
        nc, in_maps, list(range(NCORES)),
        trace=_trace, **(_trace_kwargs or {}),
    )
    # per-core out [2, B, 16, 256]: col x = 2m + parity -> (B, O, y, x)
    rows = []
    for r in range(NCORES):
        o = np.asarray(res.results[r]["out"], dtype=np.float32)
        full = np.zeros((B, W, RPC, O), np.float32)
        full[:, 0::2] = o[0].reshape(B, W // 2, RPC, O)
        full[:, 1::2] = o[1].reshape(B, W // 2, RPC, O)
        rows.append(full.transpose(0, 3, 2, 1))  # (B, O, y, x)
    out = np.concatenate(rows, axis=2)  # (B, O, H, W)
    if np.any(bias):
        out = out + bias.reshape(1, O, H, W)
    if _trace:
        _CACHE["last_result"] = res
    return np.ascontiguousarray(out.astype(np.float32))


# revision 16
# speedup vs baseline: 1.2062x; 1.0127x over previous
"""Locally-connected layer (unshared 3x3 conv, torch-unfold semantics) on 8 trn2 cores.

out[b,o,y,x] = sum_{c,i,j} weight[o, c*9+i*3+j, y*32+x] * xpad[b, c, y+i, x+j] + bias[o, l]

Sharding: spatial over L - core r owns image rows [4r, 4r+4) (128 pixels).

Design history:
  v1 (385 us): fp32, per-pixel N=64 matmuls, strided weight DMA -> 128 B
      packets, 10% memory-bandwidth utilization, DMA-bound.
  v2/v3 (63/69 us): bf16 + host relayout of weights into the SBUF stream
      layout (contiguous 6-18 KB DMA packets). K=128 matmuls occupy all PE
      rows -> every LDWEIGHTS serializes with the matmul stream.
  v4 (HW crash): K=64 alternating row groups, one psum group mixing base-0
      and base-64 blocks -> TRN2 erratum (mixed-base groups crash).
  v5 (verifier reject): walrus refuses tensor_tensor with two PSUM inputs.
  v6 (72 us): column-parity row groups, homogeneous psum groups. Correct,
      but the two row groups share the single moving-operand XBUS, so
      matmul streams serialize anyway; 576 matmuls x ~36 ns fixed overhead
      + 30.7 us of K=64 streaming. Also 179 us of summed DMA-engine idle
      right after each DMA's completion semaphore (24 dma_starts).
  v7: 64x64 ARRAY TILING, 4 independent tiles (T0/T2/T8/T10 = SBUF half x
      PSUM half). Four output columns in flight, one per tile, matmuls
      issued round-robin: column tiling allocates separate XBUSes so the
      4 streams + their LDWEIGHTS genuinely overlap. Each column's 18
      K=64 matmuls (N = vi*64 <= 192) form a homogeneous accumulation
      group in its own PSUM bank. PSUM evacuation splits across VectorE
      (lower half) and ScalarE (upper half). Fewer, larger DMAs
      (all weight packets >= 9 KB) to cut the per-dma_start completion
      bubbles; weights on the sync HWDGE ring, x and outputs on scalar.

Column-to-tile map per group of 4 columns (4k..4k+3):
  col 4k   -> T0: stationary+weights SBUF 0:64,    psum partitions 0:64
  col 4k+1 -> T2: SBUF 0:64,    psum 64:128
  col 4k+2 -> T8: SBUF 64:128,  psum 0:64
  col 4k+3 -> T10: SBUF 64:128, psum 64:128
tile_position is auto-derived from (lhsT.base_partition, out.base_partition).

x slab [p, xs, rp, b]: both partition halves hold the zero-padded slab
(host-duplicated; upper half feeds T8/T10). Weights host-packed into
WG[128, 8, 2, 2304]: [sbuf half, group, in-half index, stream col].
Out is staged [128=(parity, b), group, i, 256] and DMAed per half to
o_d[2, B, 16, 256] (parity-split columns), unscrambled on the host.

Per-core HBM: w 9.44 MB + x 3.34 MB + out 1.05 MB = 13.8 MB bf16.
"""

import numpy as np

B, C, O, H, W, KS = 64, 64, 64, 32, 32, 3
L = H * W
NCORES = 8
RPC = H // NCORES            # image rows per core = 4
SLABR = RPC + 2              # slab rows per core (with halo) = 6
XS = W + 2                   # padded slab width = 34
NG = W // 4                  # 8 groups of 4 columns

# output rows y served by slab row rp: y = rp - i, i in 0..2, clipped
YS = [max(0, rp - 2) for rp in range(SLABR)]
VI = [min(RPC - 1, rp) - max(0, rp - 2) + 1 for rp in range(SLABR)]

# per-column block list: (rp, j, ya, n, off); off = cumulative stream column
BLOCKS = []
_off = 0
for _rp in range(SLABR):
    for _j in range(KS):
        _n = VI[_rp] * O
        BLOCKS.append((_rp, _j, YS[_rp] * O, _n, _off))
        _off += _n
CCOLS = _off                 # 2304 weight stream cols per output column
assert CCOLS == 2304

# s = column slot in its group of 4: (sbuf half, psum half)
SHALF = [0, 0, 1, 1]
PHALF = [0, 1, 0, 1]

_CACHE = {}


def _build_nc():
    import concourse.bass as bass
    import concourse.bacc as bacc
    import concourse.tile as tile
    from concourse import mybir

    f32 = mybir.dt.float32
    bf16 = mybir.dt.bfloat16
    nc = bacc.Bacc(
        "TRN2", target_bir_lowering=False, debug=False, num_devices=NCORES
    )
    # x slab duplicated on both partition halves: [p, xs, rp, b]
    x_d = nc.dram_tensor("xf", [128, XS, SLABR, B], bf16, kind="ExternalInput")
    # weight stream: [p, group, i, ccol]; col = 4*group + 2*(p>=64) + i
    wg_d = nc.dram_tensor("wg", [128, NG, 2, CCOLS], bf16, kind="ExternalInput")
    # out, parity-split: [parity, b, m, (y, o)] for col x = 2m + parity
    o_d = nc.dram_tensor("out", [2, B, W // 2, RPC * O], bf16,
                         kind="ExternalOutput")

    with tile.TileContext(nc) as tc:
        with (
            tc.tile_pool(name="xp", bufs=1) as xpool,
            tc.tile_pool(name="wg", bufs=1) as wgpool,
            tc.tile_pool(name="ot", bufs=2) as opool,
            tc.tile_pool(name="ps", bufs=2, space=bass.MemorySpace.PSUM) as pspool,
        ):
            # All 8 weight-group tiles are resident (74 KB/partition total
            # with xf), so no pool cycling: every input DMA is emitted
            # up-front in consumption order, ping-ponged across the two
            # HWDGE rings. No input-side semaphore stalls -> the SDMA
            # engines never drain. Group 0's weights are split so the
            # first matmuls only wait on 0.3 MB.
            xf = xpool.tile([128, XS, SLABR, B], bf16)
            wgs = [
                wgpool.tile([128, 1, 2, CCOLS], bf16, name=f"wgk{g}", tag=f"w{g}")
                for g in range(NG)
            ]
            SPLIT = 576
            nc.sync.dma_start(xf[:, 0:6], x_d[:, 0:6])
            nc.scalar.dma_start(wgs[0][:, :, :, 0:SPLIT],
                                wg_d[:, 0:1, :, 0:SPLIT])
            nc.sync.dma_start(wgs[0][:, :, :, SPLIT:], wg_d[:, 0:1, :, SPLIT:])
            nc.scalar.dma_start(wgs[1][:], wg_d[:, 1:2])
            nc.sync.dma_start(xf[:, 6:18], x_d[:, 6:18])
            nc.scalar.dma_start(wgs[2][:], wg_d[:, 2:3])
            nc.sync.dma_start(wgs[3][:], wg_d[:, 3:4])
            nc.scalar.dma_start(wgs[4][:], wg_d[:, 4:5])
            nc.sync.dma_start(xf[:, 18:34], x_d[:, 18:34])
            nc.scalar.dma_start(wgs[5][:], wg_d[:, 5:6])
            nc.sync.dma_start(wgs[6][:], wg_d[:, 6:7])
            nc.scalar.dma_start(wgs[7][:], wg_d[:, 7:8])

            ot = None
            for g in range(NG):
                wgk = wgs[g]
                if g % 2 == 0:
                    ot = opool.tile([128, 2, 2, RPC * O], bf16,
                                    name=f"ot{g}", tag="ot")
                oi = g % 2
                pss = [
                    pspool.tile([128, RPC * O], f32,
                                name=f"ps{s}", tag=f"ps{s}")
                    for s in range(4)
                ]
                last = len(BLOCKS) - 1
                for k, (rp, j, ya, n, off) in enumerate(BLOCKS):
                    for s in range(4):
                        sh = slice(64 * SHALF[s], 64 * SHALF[s] + 64)
                        ph = slice(64 * PHALF[s], 64 * PHALF[s] + 64)
                        nc.tensor.matmul(
                            pss[s][ph, ya : ya + n],
                            xf[sh, 4 * g + s + j, rp, :],
                            wgk[sh, 0, s % 2, off : off + n],
                            start=(k == 0), stop=(k == last),
                        )
                # evacuate: lower psum halves on VectorE, upper on ScalarE;
                # ot partitions = column parity
                nc.vector.tensor_copy(ot[0:64, oi, 0, :], pss[0][0:64, :])
                nc.scalar.copy(ot[64:128, oi, 0, :], pss[1][64:128, :])
                nc.vector.tensor_copy(ot[0:64, oi, 1, :], pss[2][0:64, :])
                nc.scalar.copy(ot[64:128, oi, 1, :], pss[3][64:128, :])
                # stores ride the GPSIMD SWDGE ring so they never queue
                # behind input weight DMAs on the two HWDGE rings;
                # col of slot (g, i): even half {4g+2i}, odd {4g+2i+1}
                if g % 2 == 1:
                    nc.gpsimd.dma_start(
                        o_d[0, :, 2 * g - 2 : 2 * g + 2], ot[0:64])
                    nc.gpsimd.dma_start(
                        o_d[1, :, 2 * g - 2 : 2 * g + 2], ot[64:128])
    nc.compile()
    return nc


def _get_nc():
    if "nc" not in _CACHE:
        _CACHE["nc"] = _build_nc()
    return _CACHE["nc"]


def _shard_inputs(x, weight):
    from concourse import mybir

    bf16 = mybir.dt.np(mybir.dt.bfloat16)

    xpad = np.pad(x, ((0, 0), (0, 0), (1, 1), (1, 1)))  # (B, C, 34, 34)
    XF = np.zeros((NCORES, 128, XS, SLABR, B), np.float32)
    base = xpad.transpose(1, 3, 2, 0)  # (c, col, row, b)
    for rp in range(SLABR):
        # slab row rp of core r is padded row 4r+rp (8 cores)
        XF[:, 0:64, :, rp, :] = (
            base[:, :, rp : rp + 4 * NCORES : 4, :].transpose(2, 0, 1, 3)
        )
    XF[:, 64:128] = XF[:, 0:64]

    # weight stream; w6[o, c, i, j, y_img, x]
    w6 = weight.reshape(O, C, KS, KS, H, W)
    WG = np.zeros((NCORES, 128, NG, 2, CCOLS), np.float32)
    for rp, j, ya, n, off in BLOCKS:
        for yi in range(VI[rp]):
            y = YS[rp] + yi
            i = rp - y
            csl = slice(off + yi * O, off + (yi + 1) * O)
            # (O, C, R, W) -> (R, C, W, O); W -> (group, half, i2)
            src = (
                w6[:, :, i, j, y::RPC, :].transpose(2, 1, 3, 0)
                .reshape(NCORES, C, NG, 2, 2, O)
            )
            WG[:, 0:64, :, 0, csl] = src[:, :, :, 0, 0]   # col 4g
            WG[:, 0:64, :, 1, csl] = src[:, :, :, 0, 1]   # col 4g+1
            WG[:, 64:128, :, 0, csl] = src[:, :, :, 1, 0]  # col 4g+2
            WG[:, 64:128, :, 1, csl] = src[:, :, :, 1, 1]  # col 4g+3
    XF = XF.astype(bf16)
    WG = WG.astype(bf16)
    return [{"xf": XF[r], "wg": WG[r]} for r in range(NCORES)]


def kernel(x, weight, bias, _trace=False, _trace_kwargs=None):
    from concourse.bass_utils import run_bass_kernel_spmd

    x = np.ascontiguousarray(np.asarray(x, dtype=np.float32))
    weight = np.asarray(weight, dtype=np.float32)
    bias = np.asarray(bias, dtype=np.float32)

    nc = _get_nc()
    in_maps = _shard_inputs(x, weight)
    res = run_bass_kernel_spmd(
        nc, in_maps, list(range(NCORES)),
        trace=_trace, **(_trace_kwargs or {}),
    )
    # per-core out [2, B, 16, 256]: col x = 2m + parity -> (B, O, y, x)
    rows = []
    for r in range(NCORES):
        o = np.asarray(res.results[r]["out"], dtype=np.float32)
        full = np.zeros((B, W, RPC, O), np.float32)
        full[:, 0::2] = o[0].reshape(B, W // 2, RPC, O)
        full[:, 1::2] = o[1].reshape(B, W // 2, RPC, O)
        rows.append(full.transpose(0, 3, 2, 1))  # (B, O, y, x)
    out = np.concatenate(rows, axis=2)  # (B, O, H, W)
    if np.any(bias):
        out = out + bias.reshape(1, O, H, W)
    if _trace:
        _CACHE["last_result"] = res
    return np.ascontiguousarray(out.astype(np.float32))


# revision 19
# speedup vs baseline: 1.2474x; 1.0341x over previous
"""Locally-connected layer (unshared 3x3 conv, torch-unfold semantics) on 8 trn2 cores.

out[b,o,y,x] = sum_{c,i,j} weight[o, c*9+i*3+j, y*32+x] * xpad[b, c, y+i, x+j] + bias[o, l]

Sharding: spatial over L - core r owns image rows [4r, 4r+4) (128 pixels).

Design history:
  v1 (385 us): fp32, per-pixel N=64 matmuls, strided weight DMA -> 128 B
      packets, 10% memory-bandwidth utilization, DMA-bound.
  v2/v3 (63/69 us): bf16 + host relayout of weights into the SBUF stream
      layout (contiguous 6-18 KB DMA packets). K=128 matmuls occupy all PE
      rows -> every LDWEIGHTS serializes with the matmul stream.
  v4 (HW crash): K=64 alternating row groups, one psum group mixing base-0
      and base-64 blocks -> TRN2 erratum (mixed-base groups crash).
  v5 (verifier reject): walrus refuses tensor_tensor with two PSUM inputs.
  v6 (72 us): column-parity row groups, homogeneous psum groups. Correct,
      but the two row groups share the single moving-operand XBUS, so
      matmul streams serialize anyway; 576 matmuls x ~36 ns fixed overhead
      + 30.7 us of K=64 streaming. Also 179 us of summed DMA-engine idle
      right after each DMA's completion semaphore (24 dma_starts).
  v7: 64x64 ARRAY TILING, 4 independent tiles (T0/T2/T8/T10 = SBUF half x
      PSUM half). Four output columns in flight, one per tile, matmuls
      issued round-robin: column tiling allocates separate XBUSes so the
      4 streams + their LDWEIGHTS genuinely overlap. Each column's 18
      K=64 matmuls (N = vi*64 <= 192) form a homogeneous accumulation
      group in its own PSUM bank. PSUM evacuation splits across VectorE
      (lower half) and ScalarE (upper half). Fewer, larger DMAs
      (all weight packets >= 9 KB) to cut the per-dma_start completion
      bubbles; weights on the sync HWDGE ring, x and outputs on scalar.

Column-to-tile map per group of 4 columns (4k..4k+3):
  col 4k   -> T0: stationary+weights SBUF 0:64,    psum partitions 0:64
  col 4k+1 -> T2: SBUF 0:64,    psum 64:128
  col 4k+2 -> T8: SBUF 64:128,  psum 0:64
  col 4k+3 -> T10: SBUF 64:128, psum 64:128
tile_position is auto-derived from (lhsT.base_partition, out.base_partition).

x slab [p, xs, rp, b]: both partition halves hold the zero-padded slab
(host-duplicated; upper half feeds T8/T10). Weights host-packed into
WG[128, 8, 2, 2304]: [sbuf half, group, in-half index, stream col].
Out is staged [128=(parity, b), group, i, 256] and DMAed per half to
o_d[2, B, 16, 256] (parity-split columns), unscrambled on the host.

Per-core HBM: w 9.44 MB + x 3.34 MB + out 1.05 MB = 13.8 MB bf16.
"""

import numpy as np

B, C, O, H, W, KS = 64, 64, 64, 32, 32, 3
L = H * W
NCORES = 8
RPC = H // NCORES            # image rows per core = 4
SLABR = RPC + 2              # slab rows per core (with halo) = 6
XS = W + 2                   # padded slab width = 34
NG = W // 4                  # 8 groups of 4 columns

# output rows y served by slab row rp: y = rp - i, i in 0..2, clipped
YS = [max(0, rp - 2) for rp in range(SLABR)]
VI = [min(RPC - 1, rp) - max(0, rp - 2) + 1 for rp in range(SLABR)]

# per-column block list: (rp, j, ya, n, off); off = cumulative stream column
BLOCKS = []
_off = 0
for _rp in range(SLABR):
    for _j in range(KS):
        _n = VI[_rp] * O
        BLOCKS.append((_rp, _j, YS[_rp] * O, _n, _off))
        _off += _n
CCOLS = _off                 # 2304 weight stream cols per output column
assert CCOLS == 2304

# s = column slot in its group of 4: (sbuf half, psum half)
SHALF = [0, 0, 1, 1]
PHALF = [0, 1, 0, 1]

_CACHE = {}


def _build_nc():
    import concourse.bass as bass
    import concourse.bacc as bacc
    import concourse.tile as tile
    from concourse import mybir

    f32 = mybir.dt.float32
    bf16 = mybir.dt.bfloat16
    nc = bacc.Bacc(
        "TRN2", target_bir_lowering=False, debug=False, num_devices=NCORES
    )
    # x slab duplicated on both partition halves: [p, xs, rp, b]
    x_d = nc.dram_tensor("xf", [128, XS, SLABR, B], bf16, kind="ExternalInput")
    # weight stream: [p, group, i, ccol]; col = 4*group + 2*(p>=64) + i
    wg_d = nc.dram_tensor("wg", [128, NG, 2, CCOLS], bf16, kind="ExternalInput")
    # out, parity-split: [parity, b, m, (y, o)] for col x = 2m + parity
    o_d = nc.dram_tensor("out", [2, B, W // 2, RPC * O], bf16,
                         kind="ExternalOutput")

    with tile.TileContext(nc) as tc:
        with (
            tc.tile_pool(name="xp", bufs=1) as xpool,
            tc.tile_pool(name="wg", bufs=1) as wgpool,
            tc.tile_pool(name="ot", bufs=4) as opool,
            tc.tile_pool(name="ps", bufs=2, space=bass.MemorySpace.PSUM) as pspool,
        ):
            # All 8 weight-group tiles are resident (74 KB/partition total
            # with xf), so no pool cycling: every input DMA is emitted
            # up-front in consumption order, ping-ponged across the two
            # HWDGE rings. No input-side semaphore stalls -> the SDMA
            # engines never drain. Group 0's weights are split so the
            # first matmuls only wait on 0.3 MB.
            xf = xpool.tile([128, XS, SLABR, B], bf16)
            wgs = [
                wgpool.tile([128, 1, 2, CCOLS], bf16, name=f"wgk{g}", tag=f"w{g}")
                for g in range(NG)
            ]
            # ALL inputs on the sync ring in strict consumption order: a
            # single ring's packets spread over all 16 SDMA engines, so one
            # ring delivers the full ~425 GB/s, and single-FIFO ordering
            # means every group's weights arrive as early as possible.
            # (Two-ring splits halve each ring's delivery rate mid-run.)
            SPLIT = 576
            nc.sync.dma_start(xf[:, 0:6], x_d[:, 0:6])
            nc.sync.dma_start(wgs[0][:, :, :, 0:SPLIT],
                              wg_d[:, 0:1, :, 0:SPLIT])
            nc.sync.dma_start(wgs[0][:, :, :, SPLIT:], wg_d[:, 0:1, :, SPLIT:])
            nc.sync.dma_start(wgs[1][:], wg_d[:, 1:2])
            nc.sync.dma_start(xf[:, 6:18], x_d[:, 6:18])
            nc.sync.dma_start(wgs[2][:], wg_d[:, 2:3])
            nc.sync.dma_start(wgs[3][:], wg_d[:, 3:4])
            nc.sync.dma_start(xf[:, 18:34], x_d[:, 18:34])
            nc.sync.dma_start(wgs[4][:], wg_d[:, 4:5])
            nc.sync.dma_start(wgs[5][:], wg_d[:, 5:6])
            nc.sync.dma_start(wgs[6][:], wg_d[:, 6:7])
            nc.sync.dma_start(wgs[7][:], wg_d[:, 7:8])

            ot = None
            for g in range(NG):
                wgk = wgs[g]
                if g % 2 == 0:
                    ot = opool.tile([128, 2, 2, RPC * O], bf16,
                                    name=f"ot{g}", tag="ot")
                oi = g % 2
                pss = [
                    pspool.tile([128, RPC * O], f32,
                                name=f"ps{s}", tag=f"ps{s}")
                    for s in range(4)
                ]
                last = len(BLOCKS) - 1
                for k, (rp, j, ya, n, off) in enumerate(BLOCKS):
                    for s in range(4):
                        sh = slice(64 * SHALF[s], 64 * SHALF[s] + 64)
                        ph = slice(64 * PHALF[s], 64 * PHALF[s] + 64)
                        nc.tensor.matmul(
                            pss[s][ph, ya : ya + n],
                            xf[sh, 4 * g + s + j, rp, :],
                            wgk[sh, 0, s % 2, off : off + n],
                            start=(k == 0), stop=(k == last),
                        )
                # evacuate: lower psum halves on VectorE, upper on ScalarE;
                # ot partitions = column parity
                nc.vector.tensor_copy(ot[0:64, oi, 0, :], pss[0][0:64, :])
                nc.scalar.copy(ot[64:128, oi, 0, :], pss[1][64:128, :])
                nc.vector.tensor_copy(ot[0:64, oi, 1, :], pss[2][0:64, :])
                nc.scalar.copy(ot[64:128, oi, 1, :], pss[3][64:128, :])
                # stores ride the otherwise-empty scalar HWDGE ring;
                # col of slot (g, i): even half {4g+2i}, odd {4g+2i+1}
                if g % 2 == 1:
                    nc.scalar.dma_start(
                        o_d[0, :, 2 * g - 2 : 2 * g + 2], ot[0:64])
                    nc.scalar.dma_start(
                        o_d[1, :, 2 * g - 2 : 2 * g + 2], ot[64:128])
    nc.compile()
    return nc


def _get_nc():
    if "nc" not in _CACHE:
        _CACHE["nc"] = _build_nc()
    return _CACHE["nc"]


def _shard_inputs(x, weight):
    from concourse import mybir

    bf16 = mybir.dt.np(mybir.dt.bfloat16)

    xpad = np.pad(x, ((0, 0), (0, 0), (1, 1), (1, 1)))  # (B, C, 34, 34)
    XF = np.zeros((NCORES, 128, XS, SLABR, B), np.float32)
    base = xpad.transpose(1, 3, 2, 0)  # (c, col, row, b)
    for rp in range(SLABR):
        # slab row rp of core r is padded row 4r+rp (8 cores)
        XF[:, 0:64, :, rp, :] = (
            base[:, :, rp : rp + 4 * NCORES : 4, :].transpose(2, 0, 1, 3)
        )
    XF[:, 64:128] = XF[:, 0:64]

    # weight stream; w6[o, c, i, j, y_img, x]
    w6 = weight.reshape(O, C, KS, KS, H, W)
    WG = np.zeros((NCORES, 128, NG, 2, CCOLS), np.float32)
    for rp, j, ya, n, off in BLOCKS:
        for yi in range(VI[rp]):
            y = YS[rp] + yi
            i = rp - y
            csl = slice(off + yi * O, off + (yi + 1) * O)
            # (O, C, R, W) -> (R, C, W, O); W -> (group, half, i2)
            src = (
                w6[:, :, i, j, y::RPC, :].transpose(2, 1, 3, 0)
                .reshape(NCORES, C, NG, 2, 2, O)
            )
            WG[:, 0:64, :, 0, csl] = src[:, :, :, 0, 0]   # col 4g
            WG[:, 0:64, :, 1, csl] = src[:, :, :, 0, 1]   # col 4g+1
            WG[:, 64:128, :, 0, csl] = src[:, :, :, 1, 0]  # col 4g+2
            WG[:, 64:128, :, 1, csl] = src[:, :, :, 1, 1]  # col 4g+3
    XF = XF.astype(bf16)
    WG = WG.astype(bf16)
    return [{"xf": XF[r], "wg": WG[r]} for r in range(NCORES)]


def kernel(x, weight, bias, _trace=False, _trace_kwargs=None):
    from concourse.bass_utils import run_bass_kernel_spmd

    x = np.ascontiguousarray(np.asarray(x, dtype=np.float32))
    weight = np.asarray(weight, dtype=np.float32)
    bias = np.asarray(bias, dtype=np.float32)

    nc = _get_nc()
    in_maps = _shard_inputs(x, weight)
    res = run_bass_kernel_spmd(
        nc, in_maps, list(range(NCORES)),
        trace=_trace, **(_trace_kwargs or {}),
    )
    # per-core out [2, B, 16, 256]: col x = 2m + parity -> (B, O, y, x)
    rows = []
    for r in range(NCORES):
        o = np.asarray(res.results[r]["out"], dtype=np.float32)
        full = np.zeros((B, W, RPC, O), np.float32)
        full[:, 0::2] = o[0].reshape(B, W // 2, RPC, O)
        full[:, 1::2] = o[1].reshape(B, W // 2, RPC, O)
        rows.append(full.transpose(0, 3, 2, 1))  # (B, O, y, x)
    out = np.concatenate(rows, axis=2)  # (B, O, H, W)
    if np.any(bias):
        out = out + bias.reshape(1, O, H, W)
    if _trace:
        _CACHE["last_result"] = res
    return np.ascontiguousarray(out.astype(np.float32))
